# revision 30
# baseline (speedup 1.0000x reference)
"""Dice + CrossEntropy loss kernel for Trainium2 (8 NeuronCores, Bass/Tile).

Problem: x (16, 8, 512, 512) f32 logits, y (16, 512, 512) int labels.
    out = dice_loss + ce_loss   (scalar f32)

Sharding: pure data parallel over the batch dim - core j handles batches
[2j, 2j+1]. All cross-core reductions are tiny and done on the host.

Device layout (per core). Work unit is a "supergroup" (sg) of 65536
pixels of one batch image. SBUF tiles are [128, 4096]: partition row r
holds pixels [512r, 512r+512) of the sg; the free dim is (c, n) with
class c in 0..7 OUTER (512-column blocks) and pixel n INNER. With the
class dim in the free axis, the per-pixel quantities (1/sumexp, labels)
are read via step-0 replicated APs directly by the DVE - no partition
broadcasts, no DMA amplification.

  ACT : e8 = exp(x8)                                    [128,4096] bf16
  DVE : s = 3-level column-block add tree over c        [128,512]
  ACT : lns = ln(s); r = exp(-lns) = 1/s                [128,512]
  DVE : mask8 = (y_rep0 == ct); p8 = e8 * r_rep0        [128,4096]
  POOL: me8 = p8 * mask8                                [128,4096]
  DVE : pay = 3-level add tree over me8 blocks          [128,512]
  ACT : ln(pay) with accum_out -> CE partial column
  PE  : per class c: ones-column lhsT matmuls accumulate column sums of
        me8 block c (-> tp) and p8 block c (-> p_sum) in PSUM per batch
Host: tiny final reductions + dice/CE formula; label counts via bincount.
"""

import os
import sys

if os.path.isdir("/opt/trn_rl_repo") and "/opt/trn_rl_repo" not in sys.path:
    sys.path.insert(0, "/opt/trn_rl_repo")

import numpy as np
import ml_dtypes

B, C, H, W = 16, 8, 512, 512
HW = H * W
N_CORES = 8
B_LOC = B // N_CORES
SMOOTH = 1e-05
EPS = 1e-08

NCOLS = 512                     # pixels per partition row per sg
SGCOLS = C * NCOLS              # 4096 free dim = (c, n)
PIX_PER_SG = 128 * NCOLS        # 65536
_BF16 = ml_dtypes.bfloat16

_cache = {}


def _patch_act_tables():
    """Pin every activation to the one table set containing both Exp and
    Ln so the kernel needs a single ACT_TABLE_LOAD (set ids preserved)."""
    from concourse import hw_specs
    import concourse.bacc as bacc_mod

    if getattr(hw_specs, "_act_tables_patched", False):
        return
    orig = hw_specs.get_activation_tables

    def patched(arch):
        tables = orig(arch)
        keep = "natural_log_exp_and_others"
        if keep in tables:
            tables = {
                name: (funcs if name == keep else set())
                for name, funcs in tables.items()
            }
        return tables

    hw_specs.get_activation_tables = patched
    bacc_mod.get_activation_tables = patched
    hw_specs._act_tables_patched = True


def _build_graph(b_loc=B_LOC, hw=HW):
    _patch_act_tables()
    import concourse.bass as bass_mod
    import concourse.bacc as bacc
    import concourse.tile as tile
    from concourse import mybir

    sg_per_b = hw // PIX_PER_SG
    n_sg = b_loc * sg_per_b

    nc = bacc.Bacc()
    x_d = nc.dram_tensor("x", [b_loc, sg_per_b, 128, SGCOLS],
                         mybir.dt.bfloat16, kind="ExternalInput")
    m_d = nc.dram_tensor("m", [b_loc, sg_per_b, 128, SGCOLS],
                         mybir.dt.bfloat16, kind="ExternalInput")
    xg_d = nc.dram_tensor("xg", [b_loc, sg_per_b, 128, NCOLS],
                          mybir.dt.bfloat16, kind="ExternalInput")
    w_d = nc.dram_tensor("w", [128, 8 * C], mybir.dt.bfloat16,
                         kind="ExternalInput")
    o_tp = nc.dram_tensor("o_tp", [8 * b_loc, NCOLS], mybir.dt.float32,
                          kind="ExternalOutput")
    o_ps = nc.dram_tensor("o_ps", [8 * b_loc, NCOLS], mybir.dt.float32,
                          kind="ExternalOutput")
    o_lp = nc.dram_tensor("o_lp", [128, n_sg], mybir.dt.float32,
                          kind="ExternalOutput")

    fp32 = mybir.dt.float32
    bf16 = mybir.dt.bfloat16
    Act = mybir.ActivationFunctionType
    Alu = mybir.AluOpType

    def rep0(ap, n):
        """Insert a step-0 (replication) free dim after the partition dim."""
        return bass_mod.AP(
            tensor=ap.tensor, offset=ap.offset,
            ap=[list(ap.ap[0])] + [[0, n]] + [list(p) for p in ap.ap[1:]])

    with tile.TileContext(nc) as tc:
        with (
            tc.tile_pool(name="singles", bufs=1) as singles,
            tc.tile_pool(name="xin", bufs=4) as xin,
            tc.tile_pool(name="ybuf", bufs=3) as ybuf,
            tc.tile_pool(name="ebuf", bufs=3) as ebuf,
            tc.tile_pool(name="work", bufs=3) as work,
            tc.tile_pool(name="small", bufs=6) as small,
            tc.tile_pool(name="psB", bufs=3, space="PSUM") as psB,
            tc.tile_pool(name="psC", bufs=3, space="PSUM") as psC,
        ):
            w_sb = singles.tile([128, 8 * C], bf16)
            nc.sync.dma_start(out=w_sb, in_=w_d[:, :])
            acc_lp = singles.tile([128, n_sg], fp32)
            acc_tp = [singles.tile([8, NCOLS], fp32, name=f"acc_tp{b}")
                      for b in range(b_loc)]
            acc_ps = [singles.tile([8, NCOLS], fp32, name=f"acc_ps{b}")
                      for b in range(b_loc)]

            tp_ps = None
            ps_ps = None
            for sgi in range(n_sg):
                b = sgi // sg_per_b
                sg = sgi % sg_per_b

                xt = xin.tile([128, SGCOLS], bf16)
                nc.sync.dma_start(out=xt, in_=x_d[b, sg])
                mask8 = work.tile([128, SGCOLS], bf16, tag="mask8")
                nc.sync.dma_start(out=mask8, in_=m_d[b, sg])
                xg = ybuf.tile([128, NCOLS], bf16, tag="xg")
                nc.sync.dma_start(out=xg, in_=xg_d[b, sg])

                # e8 = exp(x)
                e8 = ebuf.tile([128, SGCOLS], bf16)
                nc.scalar.activation(e8, xt, Act.Exp)

                # s = sumexp per pixel: 3-level column-block add tree
                t1 = small.tile([128, SGCOLS // 2], bf16, tag="t1")
                nc.vector.tensor_add(t1, e8[:, 0:SGCOLS // 2],
                                     e8[:, SGCOLS // 2:SGCOLS])
                t2 = small.tile([128, SGCOLS // 4], bf16, tag="t2")
                nc.vector.tensor_add(t2, t1[:, 0:SGCOLS // 4],
                                     t1[:, SGCOLS // 4:SGCOLS // 2])
                s8 = small.tile([128, NCOLS], bf16, tag="s8")
                nc.vector.tensor_add(s8, t2[:, 0:NCOLS], t2[:, NCOLS:2 * NCOLS])

                # r = 1/s via exp(-ln)
                lns = small.tile([128, NCOLS], bf16, tag="lns")
                nc.scalar.activation(lns, s8, Act.Ln)
                r8 = small.tile([128, NCOLS], bf16, tag="r8")
                nc.scalar.activation(r8, lns, Act.Exp, scale=-1.0)

                # p = e * r  (step-0 class replication)
                p8 = work.tile([128, SGCOLS], bf16, tag="p8")
                nc.vector.tensor_tensor(
                    p8.rearrange("p (c n) -> p c n", c=C),
                    e8.rearrange("p (c n) -> p c n", c=C),
                    rep0(r8, C), Alu.mult)

                # me = p * mask
                me8 = work.tile([128, SGCOLS], bf16, tag="me8")
                nc.vector.tensor_mul(me8, p8, mask8)

                # CE partial: nll = ln(s) - x@y  (x@y gathered on host),
                # summed over the free dim into acc_lp[:, sgi]
                nll = small.tile([128, NCOLS], bf16, tag="nll")
                nc.gpsimd.tensor_sub(nll, lns, xg)
                nll_s = small.tile([128, NCOLS], bf16, tag="nll_s")
                nc.scalar.activation(nll_s, nll, Act.Identity,
                                     accum_out=acc_lp[:, sgi:sgi + 1])

                # tp / p_sum: per-class column sums accumulated in PSUM
                if sg == 0:
                    tp_ps = psB.tile([8, NCOLS], fp32, tag="tp_ps")
                    ps_ps = psC.tile([8, NCOLS], fp32, tag="ps_ps")
                for c in range(C):
                    first = (sg == 0 and c == 0)
                    last = (sg == sg_per_b - 1 and c == C - 1)
                    nc.tensor.matmul(
                        tp_ps, w_sb[:, 8 * c:8 * (c + 1)],
                        me8[:, NCOLS * c:NCOLS * (c + 1)],
                        start=first, stop=last)
                    nc.tensor.matmul(
                        ps_ps, w_sb[:, 8 * c:8 * (c + 1)],
                        p8[:, NCOLS * c:NCOLS * (c + 1)],
                        start=first, stop=last)

                if sg == sg_per_b - 1:
                    nc.vector.tensor_copy(acc_tp[b], tp_ps)
                    nc.vector.tensor_copy(acc_ps[b], ps_ps)

            for b in range(b_loc):
                nc.sync.dma_start(out=o_tp[8 * b:8 * b + 8, :], in_=acc_tp[b])
                nc.sync.dma_start(out=o_ps[8 * b:8 * b + 8, :], in_=acc_ps[b])
            nc.sync.dma_start(out=o_lp[:, :], in_=acc_lp)

    nc.finalize()
    return nc


def _host_constants():
    w = np.zeros((128, 8 * C), dtype=_BF16)
    for c in range(C):
        w[:, 8 * c + c] = 1
    return w


def _prep_xg(x_bf, y_int, hw):
    sg_per_b = hw // PIX_PER_SG
    nb = x_bf.shape[0]
    xg = np.take_along_axis(
        x_bf, y_int[:, None, :].astype(np.int64), axis=1)[:, 0]  # (nb, hw)
    return np.ascontiguousarray(xg.reshape(nb, sg_per_b, 128, NCOLS))


def _prep_mask(y_int, hw):
    sg_per_b = hw // PIX_PER_SG
    nb = y_int.shape[0]
    onehot = (y_int[:, None, :] ==
              np.arange(C, dtype=y_int.dtype)[None, :, None])  # (nb, C, hw)
    m = onehot.reshape(nb, C, sg_per_b, 128, NCOLS)
    return np.ascontiguousarray(
        m.transpose(0, 2, 3, 1, 4)).astype(_BF16).reshape(
        nb, sg_per_b, 128, SGCOLS)


def _prep_x(x, hw):
    sg_per_b = hw // PIX_PER_SG
    nb = x.shape[0]
    xr = x.reshape(nb, C, sg_per_b, 128, NCOLS)
    return np.ascontiguousarray(
        xr.transpose(0, 2, 3, 1, 4)).reshape(nb, sg_per_b, 128, SGCOLS)





def kernel(x, y):
    from concourse.bass_utils import run_bass_kernel_spmd

    x = np.asarray(x, dtype=np.float32).reshape(B, C, HW).astype(_BF16)
    y_int = np.asarray(y).reshape(B, HW)

    if "nc" not in _cache:
        _cache["nc"] = _build_graph()
    nc = _cache["nc"]

    w = _host_constants()
    x_dev = _prep_x(x, HW)
    m_dev = _prep_mask(y_int, HW)
    xg_dev = _prep_xg(x, y_int, HW)
    in_maps = [
        {
            "x": x_dev[j * B_LOC:(j + 1) * B_LOC],
            "m": m_dev[j * B_LOC:(j + 1) * B_LOC],
            "xg": xg_dev[j * B_LOC:(j + 1) * B_LOC],
            "w": w,
        }
        for j in range(N_CORES)
    ]
    res = run_bass_kernel_spmd(nc, in_maps, core_ids=list(range(N_CORES)))

    counts = np.stack(
        [np.bincount(y_int[b].astype(np.int64), minlength=C) for b in range(B)]
    ).astype(np.float64)

    tp = np.zeros((B, C), dtype=np.float64)
    ps = np.zeros((B, C), dtype=np.float64)
    lp_total = 0.0
    for j in range(N_CORES):
        r = res.results[j]
        otp = np.asarray(r["o_tp"], dtype=np.float64)
        ops_ = np.asarray(r["o_ps"], dtype=np.float64)
        olp = np.asarray(r["o_lp"], dtype=np.float64)
        for bl in range(B_LOC):
            bg = j * B_LOC + bl
            tp[bg] = otp[8 * bl:8 * bl + 8].sum(axis=1)
            ps[bg] = ops_[8 * bl:8 * bl + 8].sum(axis=1)
        lp_total += olp.sum()

    dc = (2.0 * tp + SMOOTH) / (ps + counts + SMOOTH + EPS)
    dc_loss = 1.0 - dc[:, 1:].mean()
    ce_loss = lp_total / (B * HW)
    return np.float32(dc_loss + ce_loss)


# revision 31
# speedup vs baseline: 1.0470x; 1.0470x over previous
"""Dice + CrossEntropy loss kernel for Trainium2 (8 NeuronCores, Bass/Tile).

Problem: x (16, 8, 512, 512) f32 logits, y (16, 512, 512) int labels.
    out = dice_loss + ce_loss   (scalar f32)

Sharding: pure data parallel over the batch dim - core j handles batches
[2j, 2j+1]. All cross-core reductions are tiny and done on the host.

Device layout (per core). Work unit is a "supergroup" (sg) of 65536
pixels of one batch image. SBUF tiles are [128, 4096]: partition row r
holds pixels [512r, 512r+512) of the sg; the free dim is (c, n) with
class c in 0..7 OUTER (512-column blocks) and pixel n INNER. With the
class dim in the free axis, the per-pixel quantities (1/sumexp, labels)
are read via step-0 replicated APs directly by the DVE - no partition
broadcasts, no DMA amplification.

  ACT : e8 = exp(x8)                                    [128,4096] bf16
  DVE : s = 3-level column-block add tree over c        [128,512]
  ACT : lns = ln(s); r = exp(-lns) = 1/s                [128,512]
  DVE : mask8 = (y_rep0 == ct); p8 = e8 * r_rep0        [128,4096]
  POOL: me8 = p8 * mask8                                [128,4096]
  DVE : pay = 3-level add tree over me8 blocks          [128,512]
  ACT : ln(pay) with accum_out -> CE partial column
  PE  : per class c: ones-column lhsT matmuls accumulate column sums of
        me8 block c (-> tp) and p8 block c (-> p_sum) in PSUM per batch
Host: tiny final reductions + dice/CE formula; label counts via bincount.
"""

import os
import sys

if os.path.isdir("/opt/trn_rl_repo") and "/opt/trn_rl_repo" not in sys.path:
    sys.path.insert(0, "/opt/trn_rl_repo")

import numpy as np
import ml_dtypes

B, C, H, W = 16, 8, 512, 512
HW = H * W
N_CORES = 8
B_LOC = B // N_CORES
SMOOTH = 1e-05
EPS = 1e-08

NCOLS = 512                     # pixels per partition row per sg
SGCOLS = C * NCOLS              # 4096 free dim = (c, n)
PIX_PER_SG = 128 * NCOLS        # 65536
_BF16 = ml_dtypes.bfloat16

_cache = {}


def _patch_act_tables():
    """Pin every activation to the one table set containing both Exp and
    Ln so the kernel needs a single ACT_TABLE_LOAD (set ids preserved)."""
    from concourse import hw_specs
    import concourse.bacc as bacc_mod

    if getattr(hw_specs, "_act_tables_patched", False):
        return
    orig = hw_specs.get_activation_tables

    def patched(arch):
        tables = orig(arch)
        keep = "natural_log_exp_and_others"
        if keep in tables:
            tables = {
                name: (funcs if name == keep else set())
                for name, funcs in tables.items()
            }
        return tables

    hw_specs.get_activation_tables = patched
    bacc_mod.get_activation_tables = patched
    hw_specs._act_tables_patched = True


def _build_graph(b_loc=B_LOC, hw=HW):
    _patch_act_tables()
    import concourse.bass as bass_mod
    import concourse.bacc as bacc
    import concourse.tile as tile
    from concourse import mybir

    sg_per_b = hw // PIX_PER_SG
    n_sg = b_loc * sg_per_b

    nc = bacc.Bacc()
    x_d = nc.dram_tensor("x", [b_loc, sg_per_b, 128, SGCOLS],
                         mybir.dt.bfloat16, kind="ExternalInput")
    m_d = nc.dram_tensor("m", [b_loc, sg_per_b, 128, SGCOLS],
                         mybir.dt.bfloat16, kind="ExternalInput")
    xg_d = nc.dram_tensor("xg", [b_loc, sg_per_b, 128, NCOLS],
                          mybir.dt.bfloat16, kind="ExternalInput")
    w_d = nc.dram_tensor("w", [128, 8 * C], mybir.dt.bfloat16,
                         kind="ExternalInput")
    o_tp = nc.dram_tensor("o_tp", [8 * b_loc, NCOLS], mybir.dt.float32,
                          kind="ExternalOutput")
    o_ps = nc.dram_tensor("o_ps", [8 * b_loc, NCOLS], mybir.dt.float32,
                          kind="ExternalOutput")
    o_lp = nc.dram_tensor("o_lp", [128, n_sg], mybir.dt.float32,
                          kind="ExternalOutput")

    fp32 = mybir.dt.float32
    bf16 = mybir.dt.bfloat16
    Act = mybir.ActivationFunctionType
    Alu = mybir.AluOpType

    def rep0(ap, n):
        """Insert a step-0 (replication) free dim after the partition dim."""
        return bass_mod.AP(
            tensor=ap.tensor, offset=ap.offset,
            ap=[list(ap.ap[0])] + [[0, n]] + [list(p) for p in ap.ap[1:]])

    with tile.TileContext(nc) as tc:
        with (
            tc.tile_pool(name="singles", bufs=1) as singles,
            tc.tile_pool(name="xin", bufs=4) as xin,
            tc.tile_pool(name="ybuf", bufs=3) as ybuf,
            tc.tile_pool(name="ebuf", bufs=3) as ebuf,
            tc.tile_pool(name="work", bufs=3) as work,
            tc.tile_pool(name="small", bufs=6) as small,
            tc.tile_pool(name="psB", bufs=3, space="PSUM") as psB,
            tc.tile_pool(name="psC", bufs=3, space="PSUM") as psC,
        ):
            w_sb = singles.tile([128, 8 * C], bf16)
            nc.sync.dma_start(out=w_sb, in_=w_d[:, :])
            acc_lp = singles.tile([128, n_sg], fp32)
            acc_tp = [singles.tile([8, NCOLS], fp32, name=f"acc_tp{b}")
                      for b in range(b_loc)]
            acc_ps = [singles.tile([8, NCOLS], fp32, name=f"acc_ps{b}")
                      for b in range(b_loc)]

            tp_ps = None
            ps_ps = None
            for sgi in range(n_sg):
                b = sgi // sg_per_b
                sg = sgi % sg_per_b

                xt = xin.tile([128, SGCOLS], bf16)
                nc.sync.dma_start(out=xt, in_=x_d[b, sg])
                mask8 = work.tile([128, SGCOLS], bf16, tag="mask8")
                nc.sync.dma_start(out=mask8, in_=m_d[b, sg])
                xg = ybuf.tile([128, NCOLS], bf16, tag="xg")
                nc.sync.dma_start(out=xg, in_=xg_d[b, sg])

                # e8 = exp(x)
                e8 = ebuf.tile([128, SGCOLS], bf16)
                nc.scalar.activation(e8, xt, Act.Exp)

                # s = sumexp per pixel: 3-level column-block add tree
                t1 = small.tile([128, SGCOLS // 2], bf16, tag="t1")
                nc.vector.tensor_add(t1, e8[:, 0:SGCOLS // 2],
                                     e8[:, SGCOLS // 2:SGCOLS])
                t2 = small.tile([128, SGCOLS // 4], bf16, tag="t2")
                nc.vector.tensor_add(t2, t1[:, 0:SGCOLS // 4],
                                     t1[:, SGCOLS // 4:SGCOLS // 2])
                s8 = small.tile([128, NCOLS], bf16, tag="s8")
                nc.vector.tensor_add(s8, t2[:, 0:NCOLS], t2[:, NCOLS:2 * NCOLS])

                # r = 1/s via exp(-ln)
                lns = small.tile([128, NCOLS], bf16, tag="lns")
                nc.scalar.activation(lns, s8, Act.Ln)
                r8 = small.tile([128, NCOLS], bf16, tag="r8")
                nc.scalar.activation(r8, lns, Act.Exp, scale=-1.0)

                # p = e * r  (step-0 class replication)
                p8 = work.tile([128, SGCOLS], bf16, tag="p8")
                nc.vector.tensor_tensor(
                    p8.rearrange("p (c n) -> p c n", c=C),
                    e8.rearrange("p (c n) -> p c n", c=C),
                    rep0(r8, C), Alu.mult)

                # me = p * mask
                me8 = work.tile([128, SGCOLS], bf16, tag="me8")
                nc.vector.tensor_mul(me8, p8, mask8)

                # CE partial: nll = ln(s) - x@y  (x@y gathered on host),
                # summed over the free dim into acc_lp[:, sgi]
                nll = small.tile([128, NCOLS], bf16, tag="nll")
                nc.vector.tensor_sub(nll, lns, xg)
                nll_s = small.tile([128, NCOLS], bf16, tag="nll_s")
                nc.scalar.activation(nll_s, nll, Act.Identity,
                                     accum_out=acc_lp[:, sgi:sgi + 1])

                # tp / p_sum: per-class column sums accumulated in PSUM
                if sg == 0:
                    tp_ps = psB.tile([8, NCOLS], fp32, tag="tp_ps")
                    ps_ps = psC.tile([8, NCOLS], fp32, tag="ps_ps")
                for c in range(C):
                    first = (sg == 0 and c == 0)
                    last = (sg == sg_per_b - 1 and c == C - 1)
                    nc.tensor.matmul(
                        tp_ps, w_sb[:, 8 * c:8 * (c + 1)],
                        me8[:, NCOLS * c:NCOLS * (c + 1)],
                        start=first, stop=last)
                    nc.tensor.matmul(
                        ps_ps, w_sb[:, 8 * c:8 * (c + 1)],
                        p8[:, NCOLS * c:NCOLS * (c + 1)],
                        start=first, stop=last)

                if sg == sg_per_b - 1:
                    nc.vector.tensor_copy(acc_tp[b], tp_ps)
                    nc.vector.tensor_copy(acc_ps[b], ps_ps)

            for b in range(b_loc):
                nc.sync.dma_start(out=o_tp[8 * b:8 * b + 8, :], in_=acc_tp[b])
                nc.sync.dma_start(out=o_ps[8 * b:8 * b + 8, :], in_=acc_ps[b])
            nc.sync.dma_start(out=o_lp[:, :], in_=acc_lp)

    nc.finalize()
    return nc


def _host_constants():
    w = np.zeros((128, 8 * C), dtype=_BF16)
    for c in range(C):
        w[:, 8 * c + c] = 1
    return w


def _prep_xg(x_bf, y_int, hw):
    sg_per_b = hw // PIX_PER_SG
    nb = x_bf.shape[0]
    xg = np.take_along_axis(
        x_bf, y_int[:, None, :].astype(np.int64), axis=1)[:, 0]  # (nb, hw)
    return np.ascontiguousarray(xg.reshape(nb, sg_per_b, 128, NCOLS))


def _prep_mask(y_int, hw):
    sg_per_b = hw // PIX_PER_SG
    nb = y_int.shape[0]
    onehot = (y_int[:, None, :] ==
              np.arange(C, dtype=y_int.dtype)[None, :, None])  # (nb, C, hw)
    m = onehot.reshape(nb, C, sg_per_b, 128, NCOLS)
    return np.ascontiguousarray(
        m.transpose(0, 2, 3, 1, 4)).astype(_BF16).reshape(
        nb, sg_per_b, 128, SGCOLS)


def _prep_x(x, hw):
    sg_per_b = hw // PIX_PER_SG
    nb = x.shape[0]
    xr = x.reshape(nb, C, sg_per_b, 128, NCOLS)
    return np.ascontiguousarray(
        xr.transpose(0, 2, 3, 1, 4)).reshape(nb, sg_per_b, 128, SGCOLS)





def kernel(x, y):
    from concourse.bass_utils import run_bass_kernel_spmd

    x = np.asarray(x, dtype=np.float32).reshape(B, C, HW).astype(_BF16)
    y_int = np.asarray(y).reshape(B, HW)

    if "nc" not in _cache:
        _cache["nc"] = _build_graph()
    nc = _cache["nc"]

    w = _host_constants()
    x_dev = _prep_x(x, HW)
    m_dev = _prep_mask(y_int, HW)
    xg_dev = _prep_xg(x, y_int, HW)
    in_maps = [
        {
            "x": x_dev[j * B_LOC:(j + 1) * B_LOC],
            "m": m_dev[j * B_LOC:(j + 1) * B_LOC],
            "xg": xg_dev[j * B_LOC:(j + 1) * B_LOC],
            "w": w,
        }
        for j in range(N_CORES)
    ]
    res = run_bass_kernel_spmd(nc, in_maps, core_ids=list(range(N_CORES)))

    counts = np.stack(
        [np.bincount(y_int[b].astype(np.int64), minlength=C) for b in range(B)]
    ).astype(np.float64)

    tp = np.zeros((B, C), dtype=np.float64)
    ps = np.zeros((B, C), dtype=np.float64)
    lp_total = 0.0
    for j in range(N_CORES):
        r = res.results[j]
        otp = np.asarray(r["o_tp"], dtype=np.float64)
        ops_ = np.asarray(r["o_ps"], dtype=np.float64)
        olp = np.asarray(r["o_lp"], dtype=np.float64)
        for bl in range(B_LOC):
            bg = j * B_LOC + bl
            tp[bg] = otp[8 * bl:8 * bl + 8].sum(axis=1)
            ps[bg] = ops_[8 * bl:8 * bl + 8].sum(axis=1)
        lp_total += olp.sum()

    dc = (2.0 * tp + SMOOTH) / (ps + counts + SMOOTH + EPS)
    dc_loss = 1.0 - dc[:, 1:].mean()
    ce_loss = lp_total / (B * HW)
    return np.float32(dc_loss + ce_loss)


# revision 32
# speedup vs baseline: 1.0498x; 1.0026x over previous
"""Dice + CrossEntropy loss kernel for Trainium2 (8 NeuronCores, Bass/Tile).

Problem: x (16, 8, 512, 512) f32 logits, y (16, 512, 512) int labels.
    out = dice_loss + ce_loss   (scalar f32)

Sharding: pure data parallel over the batch dim - core j handles batches
[2j, 2j+1]. All cross-core reductions are tiny and done on the host.

Device layout (per core). Work unit is a "supergroup" (sg) of 65536
pixels of one batch image. SBUF tiles are [128, 4096]: partition row r
holds pixels [512r, 512r+512) of the sg; the free dim is (c, n) with
class c in 0..7 OUTER (512-column blocks) and pixel n INNER. With the
class dim in the free axis, the per-pixel quantities (1/sumexp, labels)
are read via step-0 replicated APs directly by the DVE - no partition
broadcasts, no DMA amplification.

  ACT : e8 = exp(x8)                                    [128,4096] bf16
  DVE : s = 3-level column-block add tree over c        [128,512]
  ACT : lns = ln(s); r = exp(-lns) = 1/s                [128,512]
  DVE : mask8 = (y_rep0 == ct); p8 = e8 * r_rep0        [128,4096]
  POOL: me8 = p8 * mask8                                [128,4096]
  DVE : pay = 3-level add tree over me8 blocks          [128,512]
  ACT : ln(pay) with accum_out -> CE partial column
  PE  : per class c: ones-column lhsT matmuls accumulate column sums of
        me8 block c (-> tp) and p8 block c (-> p_sum) in PSUM per batch
Host: tiny final reductions + dice/CE formula; label counts via bincount.
"""

import os
import sys

if os.path.isdir("/opt/trn_rl_repo") and "/opt/trn_rl_repo" not in sys.path:
    sys.path.insert(0, "/opt/trn_rl_repo")

import numpy as np
import ml_dtypes

B, C, H, W = 16, 8, 512, 512
HW = H * W
N_CORES = 8
B_LOC = B // N_CORES
SMOOTH = 1e-05
EPS = 1e-08

NCOLS = 512                     # pixels per partition row per sg
SGCOLS = C * NCOLS              # 4096 free dim = (c, n)
PIX_PER_SG = 128 * NCOLS        # 65536
_BF16 = ml_dtypes.bfloat16

_cache = {}


def _patch_act_tables():
    """Pin every activation to the one table set containing both Exp and
    Ln so the kernel needs a single ACT_TABLE_LOAD (set ids preserved)."""
    from concourse import hw_specs
    import concourse.bacc as bacc_mod

    if getattr(hw_specs, "_act_tables_patched", False):
        return
    orig = hw_specs.get_activation_tables

    def patched(arch):
        tables = orig(arch)
        keep = "natural_log_exp_and_others"
        if keep in tables:
            tables = {
                name: (funcs if name == keep else set())
                for name, funcs in tables.items()
            }
        return tables

    hw_specs.get_activation_tables = patched
    bacc_mod.get_activation_tables = patched
    hw_specs._act_tables_patched = True


def _build_graph(b_loc=B_LOC, hw=HW):
    _patch_act_tables()
    import concourse.bass as bass_mod
    import concourse.bacc as bacc
    import concourse.tile as tile
    from concourse import mybir

    sg_per_b = hw // PIX_PER_SG
    n_sg = b_loc * sg_per_b

    nc = bacc.Bacc()
    x_d = nc.dram_tensor("x", [b_loc, sg_per_b, 128, SGCOLS],
                         mybir.dt.bfloat16, kind="ExternalInput")
    m_d = nc.dram_tensor("m", [b_loc, sg_per_b, 128, SGCOLS],
                         mybir.dt.bfloat16, kind="ExternalInput")
    xg_d = nc.dram_tensor("xg", [b_loc, sg_per_b, 128, NCOLS],
                          mybir.dt.bfloat16, kind="ExternalInput")
    w_d = nc.dram_tensor("w", [128, 8 * C], mybir.dt.bfloat16,
                         kind="ExternalInput")
    o_tp = nc.dram_tensor("o_tp", [8 * b_loc, NCOLS], mybir.dt.float32,
                          kind="ExternalOutput")
    o_ps = nc.dram_tensor("o_ps", [8 * b_loc, NCOLS], mybir.dt.float32,
                          kind="ExternalOutput")
    o_lp = nc.dram_tensor("o_lp", [128, n_sg], mybir.dt.float32,
                          kind="ExternalOutput")

    fp32 = mybir.dt.float32
    bf16 = mybir.dt.bfloat16
    Act = mybir.ActivationFunctionType
    Alu = mybir.AluOpType

    def rep0(ap, n):
        """Insert a step-0 (replication) free dim after the partition dim."""
        return bass_mod.AP(
            tensor=ap.tensor, offset=ap.offset,
            ap=[list(ap.ap[0])] + [[0, n]] + [list(p) for p in ap.ap[1:]])

    with tile.TileContext(nc) as tc:
        with (
            tc.tile_pool(name="singles", bufs=1) as singles,
            tc.tile_pool(name="xin", bufs=4) as xin,
            tc.tile_pool(name="ybuf", bufs=3) as ybuf,
            tc.tile_pool(name="ebuf", bufs=3) as ebuf,
            tc.tile_pool(name="work", bufs=3) as work,
            tc.tile_pool(name="small", bufs=6) as small,
            tc.tile_pool(name="psB", bufs=3, space="PSUM") as psB,
            tc.tile_pool(name="psC", bufs=3, space="PSUM") as psC,
        ):
            w_sb = singles.tile([128, 8 * C], bf16)
            nc.sync.dma_start(out=w_sb, in_=w_d[:, :])
            acc_lp = singles.tile([128, n_sg], fp32)
            acc_tp = [singles.tile([8, NCOLS], fp32, name=f"acc_tp{b}")
                      for b in range(b_loc)]
            acc_ps = [singles.tile([8, NCOLS], fp32, name=f"acc_ps{b}")
                      for b in range(b_loc)]

            tp_ps = None
            ps_ps = None
            for sgi in range(n_sg):
                b = sgi // sg_per_b
                sg = sgi % sg_per_b

                xt = xin.tile([128, SGCOLS], bf16)
                nc.sync.dma_start(out=xt, in_=x_d[b, sg])
                mask8 = work.tile([128, SGCOLS], bf16, tag="mask8")
                nc.sync.dma_start(out=mask8, in_=m_d[b, sg])
                xg = ybuf.tile([128, NCOLS], bf16, tag="xg")
                nc.sync.dma_start(out=xg, in_=xg_d[b, sg])

                # e8 = exp(x)
                e8 = ebuf.tile([128, SGCOLS], bf16)
                nc.scalar.activation(e8, xt, Act.Exp)

                # s = sumexp per pixel: 3-level column-block add tree
                t1 = small.tile([128, SGCOLS // 2], bf16, tag="t1")
                nc.vector.tensor_add(t1, e8[:, 0:SGCOLS // 2],
                                     e8[:, SGCOLS // 2:SGCOLS])
                t2 = small.tile([128, SGCOLS // 4], bf16, tag="t2")
                nc.vector.tensor_add(t2, t1[:, 0:SGCOLS // 4],
                                     t1[:, SGCOLS // 4:SGCOLS // 2])
                s8 = small.tile([128, NCOLS], bf16, tag="s8")
                nc.vector.tensor_add(s8, t2[:, 0:NCOLS], t2[:, NCOLS:2 * NCOLS])

                # r = 1/s via exp(-ln)
                lns = small.tile([128, NCOLS], bf16, tag="lns")
                nc.scalar.activation(lns, s8, Act.Ln)
                r8 = small.tile([128, NCOLS], bf16, tag="r8")
                nc.scalar.activation(r8, lns, Act.Exp, scale=-1.0)

                # p = e * r  (step-0 class replication)
                p8 = work.tile([128, SGCOLS], bf16, tag="p8")
                nc.vector.tensor_tensor(
                    p8.rearrange("p (c n) -> p c n", c=C),
                    e8.rearrange("p (c n) -> p c n", c=C),
                    rep0(r8, C), Alu.mult)

                # p_sum matmuls first: p8 is ready before me8, so the PE
                # can start while the DVE still computes me8
                if sg == 0:
                    tp_ps = psB.tile([8, NCOLS], fp32, tag="tp_ps")
                    ps_ps = psC.tile([8, NCOLS], fp32, tag="ps_ps")
                for c in range(C):
                    nc.tensor.matmul(
                        ps_ps, w_sb[:, 8 * c:8 * (c + 1)],
                        p8[:, NCOLS * c:NCOLS * (c + 1)],
                        start=(sg == 0 and c == 0),
                        stop=(sg == sg_per_b - 1 and c == C - 1))

                # me = p * mask
                me8 = work.tile([128, SGCOLS], bf16, tag="me8")
                nc.vector.tensor_mul(me8, p8, mask8)

                # CE partial: nll = ln(s) - x@y  (x@y gathered on host),
                # summed over the free dim into acc_lp[:, sgi]
                nll = small.tile([128, NCOLS], bf16, tag="nll")
                nc.vector.tensor_sub(nll, lns, xg)
                nll_s = small.tile([128, NCOLS], bf16, tag="nll_s")
                nc.scalar.activation(nll_s, nll, Act.Identity,
                                     accum_out=acc_lp[:, sgi:sgi + 1])

                # tp: per-class column sums of me8 accumulated in PSUM
                for c in range(C):
                    nc.tensor.matmul(
                        tp_ps, w_sb[:, 8 * c:8 * (c + 1)],
                        me8[:, NCOLS * c:NCOLS * (c + 1)],
                        start=(sg == 0 and c == 0),
                        stop=(sg == sg_per_b - 1 and c == C - 1))

                if sg == sg_per_b - 1:
                    nc.vector.tensor_copy(acc_tp[b], tp_ps)
                    nc.vector.tensor_copy(acc_ps[b], ps_ps)

            for b in range(b_loc):
                nc.sync.dma_start(out=o_tp[8 * b:8 * b + 8, :], in_=acc_tp[b])
                nc.sync.dma_start(out=o_ps[8 * b:8 * b + 8, :], in_=acc_ps[b])
            nc.sync.dma_start(out=o_lp[:, :], in_=acc_lp)

    nc.finalize()
    return nc


def _host_constants():
    w = np.zeros((128, 8 * C), dtype=_BF16)
    for c in range(C):
        w[:, 8 * c + c] = 1
    return w


def _prep_xg(x_bf, y_int, hw):
    sg_per_b = hw // PIX_PER_SG
    nb = x_bf.shape[0]
    xg = np.take_along_axis(
        x_bf, y_int[:, None, :].astype(np.int64), axis=1)[:, 0]  # (nb, hw)
    return np.ascontiguousarray(xg.reshape(nb, sg_per_b, 128, NCOLS))


def _prep_mask(y_int, hw):
    sg_per_b = hw // PIX_PER_SG
    nb = y_int.shape[0]
    onehot = (y_int[:, None, :] ==
              np.arange(C, dtype=y_int.dtype)[None, :, None])  # (nb, C, hw)
    m = onehot.reshape(nb, C, sg_per_b, 128, NCOLS)
    return np.ascontiguousarray(
        m.transpose(0, 2, 3, 1, 4)).astype(_BF16).reshape(
        nb, sg_per_b, 128, SGCOLS)


def _prep_x(x, hw):
    sg_per_b = hw // PIX_PER_SG
    nb = x.shape[0]
    xr = x.reshape(nb, C, sg_per_b, 128, NCOLS)
    return np.ascontiguousarray(
        xr.transpose(0, 2, 3, 1, 4)).reshape(nb, sg_per_b, 128, SGCOLS)





def kernel(x, y):
    from concourse.bass_utils import run_bass_kernel_spmd

    x = np.asarray(x, dtype=np.float32).reshape(B, C, HW).astype(_BF16)
    y_int = np.asarray(y).reshape(B, HW)

    if "nc" not in _cache:
        _cache["nc"] = _build_graph()
    nc = _cache["nc"]

    w = _host_constants()
    x_dev = _prep_x(x, HW)
    m_dev = _prep_mask(y_int, HW)
    xg_dev = _prep_xg(x, y_int, HW)
    in_maps = [
        {
            "x": x_dev[j * B_LOC:(j + 1) * B_LOC],
            "m": m_dev[j * B_LOC:(j + 1) * B_LOC],
            "xg": xg_dev[j * B_LOC:(j + 1) * B_LOC],
            "w": w,
        }
        for j in range(N_CORES)
    ]
    res = run_bass_kernel_spmd(nc, in_maps, core_ids=list(range(N_CORES)))

    counts = np.stack(
        [np.bincount(y_int[b].astype(np.int64), minlength=C) for b in range(B)]
    ).astype(np.float64)

    tp = np.zeros((B, C), dtype=np.float64)
    ps = np.zeros((B, C), dtype=np.float64)
    lp_total = 0.0
    for j in range(N_CORES):
        r = res.results[j]
        otp = np.asarray(r["o_tp"], dtype=np.float64)
        ops_ = np.asarray(r["o_ps"], dtype=np.float64)
        olp = np.asarray(r["o_lp"], dtype=np.float64)
        for bl in range(B_LOC):
            bg = j * B_LOC + bl
            tp[bg] = otp[8 * bl:8 * bl + 8].sum(axis=1)
            ps[bg] = ops_[8 * bl:8 * bl + 8].sum(axis=1)
        lp_total += olp.sum()

    dc = (2.0 * tp + SMOOTH) / (ps + counts + SMOOTH + EPS)
    dc_loss = 1.0 - dc[:, 1:].mean()
    ce_loss = lp_total / (B * HW)
    return np.float32(dc_loss + ce_loss)
